# Initial kernel scaffold
#
"""EdgeEncoder kernel for Trainium2 (8 NeuronCores, row-sharded).

Reference (per pair (i, j) of an N x N grid):
    out[h, i, j] = (1/n_ij) * sum_l mask[i,j,l] * sum_d feats[idx[i,j,l], d] * W[l, h, d]
with n_ij = max(#valid l, 1), idx in [-1, E-1], -1 = padding.

Device strategy (per core, which owns 64 rows i):
  - Projected tables T_l[e, h] = sum_d feats[e,d] W[l,h,d] are built on PE as
    [128 channels, e] tiles: channel (16g+c): c<8 -> value column h=c, c>=8 ->
    "validity" column (constant 1 via an appended ones-feature row).  Row 0 of
    each l-block is zeros (padding target).
  - gpsimd ap_gather: Q7 core g gathers the stream (pair in share_g) from the
    SBUF-resident table; all 16 channels of the core follow the stream, so
    values for all 8 heads AND the validity bit arrive in one pass.  One phase
    per l (5 tables of 10001 rows; int16 indices), table builds overlap the
    previous phase's gather.
  - DVE reduces over l into acc[(g,c), (i_l, j)]; counts land on channels c>=8.
  - recip(max(count,1)) is aligned to the value channels by a partition-shifting
    SBUF->SBUF DMA (+8), then one multiply; output DMAs write j-contiguous runs.
"""

import numpy as np

import concourse.bass as bass
import concourse.mybir as mybir
import concourse.tile as tile
from concourse import bacc
from concourse.bass_utils import run_bass_kernel_spmd

N, L, H, D, E = 512, 5, 8, 16, 10000
NCORES = 8
RPC = N // NCORES            # 64 rows (i) per core
IPG = RPC // 8               # 8 rows (i) per Q7 core / share
PAIRS_G = IPG * N            # 4096 pairs per share
BLK = E + 1                  # 10001 rows per l-block (row 0 = zeros)
PHASES = ([0], [1], [2], [3], [4])
PCH = 2048                   # pairs per gather chunk
NCHK = PAIRS_G // PCH        # 1 chunk
ECH = 512                    # e-chunk for table build
f32, i32, i16 = mybir.dt.float32, mybir.dt.int32, mybir.dt.int16

# idxw column layout: phase p starts at IDXC[p] (int16 cols per partition)
_c = 0
IDXC = []
for ls in PHASES:
    IDXC.append(_c)
    _c += PAIRS_G * len(ls) // 16
IDXW_COLS = _c               # 1280

_cached = {}


def build_nc():
    nc = bacc.Bacc()

    idxw_t = nc.dram_tensor("idxw", [128, IDXW_COLS], i16, kind="ExternalInput")
    fw = nc.dram_tensor("fw", [D + 1, E + 5 * 128], f32, kind="ExternalInput")
    out = nc.dram_tensor("out", [H, RPC, N], f32, kind="ExternalOutput")

    with tile.TileContext(nc) as tc:
        with (
            tc.tile_pool(name="const", bufs=1) as cpool,
            tc.tile_pool(name="tbl", bufs=2) as tpool,
            tc.tile_pool(name="mm", bufs=2, space="PSUM") as mmpool,
            tc.tile_pool(name="gth", bufs=2) as gpool,
            tc.tile_pool(name="acc", bufs=1) as apool,
        ):
            # tiny dummy gather: forces the gpsimd ucode library load to
            # happen here, overlapped with the input DMAs below
            zi = cpool.tile([128, 1], i16)
            nc.vector.memset(zi[:, :], 0)
            zt = cpool.tile([128, 16], f32)
            nc.vector.memset(zt[:, :], 0.0)
            zo = cpool.tile([128, 16], f32)
            nc.gpsimd.ap_gather(
                out_ap=zo[:, :], in_ap=zt[:, :], idxs_ap=zi[:, :],
                channels=128, num_elems=16, d=1, num_idxs=16,
            )

            fw_sb = cpool.tile([D + 1, E + 5 * 128], f32)
            nc.sync.dma_start(out=fw_sb[:, :], in_=fw[:, :])
            idxw = cpool.tile([128, IDXW_COLS], i16)
            nc.sync.dma_start(out=idxw[:, :], in_=idxw_t[:, :])

            acc = apool.tile([128, PAIRS_G], f32)       # [(g,c), (il, j)]

            for p, ls in enumerate(PHASES):
                l = ls[0]
                tbl = tpool.tile([128, BLK], f32, tag="tbl")
                nc.vector.memset(tbl[:, 0:1], 0.0)
                for e0 in range(0, E, 4 * ECH):
                    bcnt = min(4 * ECH, E - e0)
                    ps = mmpool.tile([128, 4 * ECH], f32, space="PSUM", tag="mm")
                    for s0 in range(0, bcnt, ECH):
                        cnt = min(ECH, bcnt - s0)
                        nc.tensor.matmul(
                            out=ps[:, s0:s0 + cnt],
                            lhsT=fw_sb[:, E + l * 128:E + (l + 1) * 128],
                            rhs=fw_sb[:, e0 + s0:e0 + s0 + cnt],
                            start=True,
                            stop=True,
                        )
                    nc.vector.tensor_copy(
                        out=tbl[:, 1 + e0:1 + e0 + bcnt],
                        in_=ps[:, :bcnt],
                    )
                for ch in range(NCHK):
                    gth = gpool.tile([128, PCH], f32, tag="gth")
                    c0 = IDXC[p] + ch * (PCH // 16)
                    nc.gpsimd.ap_gather(
                        out_ap=gth[:, :],
                        in_ap=tbl[:, :],
                        idxs_ap=idxw[:, c0:c0 + PCH // 16],
                        channels=128,
                        num_elems=BLK,
                        d=1,
                        num_idxs=PCH,
                    )
                    asl = acc[:, ch * PCH:(ch + 1) * PCH]
                    if p == 0:
                        nc.vector.tensor_copy(out=asl, in_=gth[:, :])
                    else:
                        nc.vector.tensor_add(out=asl, in0=asl, in1=gth[:, :])

            # 1/max(count,1) lives on channels c>=8; shift to value channels
            # tail, split by gather chunk so half overlaps the last gather
            rt = gpool.tile([128, PAIRS_G], f32, tag="gth")
            scratch = gpool.tile([128, PAIRS_G], f32, tag="gth")
            rt2 = apool.tile([128, PAIRS_G], f32)
            for ch in range(NCHK):
                sl = slice(ch * PCH, (ch + 1) * PCH)
                nc.vector.tensor_scalar_max(
                    out=rt[:, sl], in0=acc[:, sl], scalar1=1.0
                )
                # ~2 ULP, ~2.8x faster than InstReciprocal (counts in [1,5])
                nc.vector.reciprocal_approx_accurate(
                    out=rt2[:, sl], in_=rt[:, sl], scratch=scratch[:, sl]
                )
                nc.sync.dma_start(out=scratch[0:120, sl], in_=rt2[8:128, sl])
                nc.vector.tensor_tensor(
                    out=rt[0:120, sl], in0=acc[0:120, sl],
                    in1=scratch[0:120, sl], op=mybir.AluOpType.mult,
                )
                ni = PCH // N  # i-rows per chunk (4)
                for g in range(8):
                    i0 = g * IPG + ch * ni
                    dst = out[:, i0:i0 + ni, :]
                    nc.sync.dma_start(
                        out=dst.rearrange("h i j -> h i j"),
                        in_=rt[16 * g:16 * g + H, sl].rearrange(
                            "c (i j) -> c i j", j=N
                        ),
                    )
    nc.compile()
    return nc


def _host_prep(edge_features_s, edge_weights, shortest_path_edges):
    feats = np.asarray(edge_features_s, dtype=np.float32)
    ew = np.asarray(edge_weights, dtype=np.float32)
    spe = np.asarray(shortest_path_edges).astype(np.int64)

    # fw = [featsT17 | W_CH]:
    #   featsT17 [17, E]: feats^T with an appended ones row
    #   W_CH [17, 5*128]: col l*128+p: p%16<8 -> (W[l, p%16, :], 0); else (0.., 1)
    W = ew[1:L + 1].reshape(L, H, D)
    featsT17 = np.concatenate([feats.T, np.ones((1, E), np.float32)], axis=0)
    wch = np.zeros((D + 1, 5 * 128), np.float32)
    for l in range(L):
        for p in range(128):
            c = p % 16
            if c < H:
                wch[:D, l * 128 + p] = W[l, c]
            else:
                wch[D, l * 128 + p] = 1.0
    fw = np.ascontiguousarray(np.concatenate([featsT17, wch], axis=1))

    comb = (spe + 1).astype(np.int32)   # [N, N, L], 0 = padding
    # per-device wrapped index streams
    idxw_all = np.zeros((NCORES, 128, IDXW_COLS), np.int16)
    for cdev in range(NCORES):
        sub = comb[cdev * RPC:(cdev + 1) * RPC]  # [64, 512, 5]
        for p, ls in enumerate(PHASES):
            npos = PAIRS_G * len(ls)
            for g in range(8):
                st = sub[g * IPG:(g + 1) * IPG][:, :, ls].astype(np.int32)
                st = st + BLK * np.arange(len(ls), dtype=np.int32)
                flat = st.reshape(-1)
                wrapped = flat.reshape(npos // 16, 16).T
                idxw_all[cdev, 16 * g:16 * g + 16,
                         IDXC[p]:IDXC[p] + npos // 16] = wrapped
    return fw, idxw_all


def kernel(edge_features_s, edge_weights, shortest_path_edges):
    if "nc" not in _cached:
        _cached["nc"] = build_nc()
    nc = _cached["nc"]

    fw, idxw_all = _host_prep(edge_features_s, edge_weights, shortest_path_edges)
    in_maps = []
    for c in range(NCORES):
        in_maps.append({
            "idxw": np.ascontiguousarray(idxw_all[c]),
            "fw": fw,
        })
    res = run_bass_kernel_spmd(nc, in_maps, list(range(NCORES)))
    outs = [res.results[c]["out"].reshape(H, RPC, N) for c in range(NCORES)]
    return np.concatenate(outs, axis=1)



# revision 1
# speedup vs baseline: 2.8398x; 2.8398x over previous
"""EdgeEncoder kernel for Trainium2 (8 NeuronCores, row-sharded).

Reference (per pair (i, j) of an N x N grid):
    out[h, i, j] = (1/n_ij) * sum_l mask[i,j,l] * sum_d feats[idx[i,j,l], d] * W[l, h, d]
with n_ij = max(#valid l, 1), idx in [-1, E-1], -1 = padding.

Device strategy (per core, which owns 64 rows i):
  - Projected tables T_l[e, h] = sum_d feats[e,d] W[l,h,d] are built on PE as
    [128 channels, e] tiles: channel (16g+c): c<8 -> value column h=c, c>=8 ->
    "validity" column (constant 1 via an appended ones-feature row).  Row 0 of
    each l-block is zeros (padding target).
  - gpsimd ap_gather: Q7 core g gathers the stream (pair in share_g) from the
    SBUF-resident table; all 16 channels of the core follow the stream, so
    values for all 8 heads AND the validity bit arrive in one pass.  One phase
    per l (5 tables of 10001 rows; int16 indices), table builds overlap the
    previous phase's gather.
  - DVE reduces over l into acc[(g,c), (i_l, j)]; counts land on channels c>=8.
  - recip(max(count,1)) is aligned to the value channels by a partition-shifting
    SBUF->SBUF DMA (+8), then one multiply; output DMAs write j-contiguous runs.
"""

import numpy as np

import concourse.bass as bass
import concourse.mybir as mybir
import concourse.tile as tile
from concourse import bacc
from concourse.bass_utils import run_bass_kernel_spmd

N, L, H, D, E = 512, 5, 8, 16, 10000
NCORES = 8
RPC = N // NCORES            # 64 rows (i) per core
IPG = RPC // 8               # 8 rows (i) per Q7 core / share
PAIRS_G = IPG * N            # 4096 pairs per share
BLK = E + 1                  # 10001 rows per l-block (row 0 = zeros)
PHASES = ([0], [1], [2], [3], [4])
PCH = 2048                   # pairs per gather chunk
NCHK = PAIRS_G // PCH        # 1 chunk
ECH = 512                    # e-chunk for table build
f32, i32, i16 = mybir.dt.float32, mybir.dt.int32, mybir.dt.int16

# idxw column layout: phase p starts at IDXC[p] (int16 cols per partition)
_c = 0
IDXC = []
for ls in PHASES:
    IDXC.append(_c)
    _c += PAIRS_G * len(ls) // 16
IDXW_COLS = _c               # 1280

_cached = {}


def build_nc():
    nc = bacc.Bacc()

    idxw_t = nc.dram_tensor("idxw", [128, IDXW_COLS], i16, kind="ExternalInput")
    fw = nc.dram_tensor("fw", [D + 1, E + 5 * 128], f32, kind="ExternalInput")
    out = nc.dram_tensor("out", [H, RPC, N], f32, kind="ExternalOutput")

    with tile.TileContext(nc) as tc:
        with (
            tc.tile_pool(name="const", bufs=1) as cpool,
            tc.tile_pool(name="tbl", bufs=2) as tpool,
            tc.tile_pool(name="mm", bufs=2, space="PSUM") as mmpool,
            tc.tile_pool(name="gth", bufs=2) as gpool,
            tc.tile_pool(name="acc", bufs=1) as apool,
        ):
            # tiny dummy gather: forces the gpsimd ucode library load to
            # happen here, overlapped with the input DMAs below
            zi = cpool.tile([128, 1], i16)
            nc.vector.memset(zi[:, :], 0)
            zt = cpool.tile([128, 16], f32)
            nc.vector.memset(zt[:, :], 0.0)
            zo = cpool.tile([128, 16], f32)
            nc.gpsimd.ap_gather(
                out_ap=zo[:, :], in_ap=zt[:, :], idxs_ap=zi[:, :],
                channels=128, num_elems=16, d=1, num_idxs=16,
            )

            fw_sb = cpool.tile([D + 1, E + 5 * 128], f32)
            nc.sync.dma_start(out=fw_sb[:, :], in_=fw[:, :])
            idxw = cpool.tile([128, IDXW_COLS], i16)
            nc.sync.dma_start(out=idxw[:, :], in_=idxw_t[:, :])

            acc = apool.tile([128, PAIRS_G], f32)       # [(g,c), (il, j)]

            for p, ls in enumerate(PHASES):
                l = ls[0]
                tbl = tpool.tile([128, BLK], f32, tag="tbl")
                nc.vector.memset(tbl[:, 0:1], 0.0)
                for e0 in range(0, E, 4 * ECH):
                    bcnt = min(4 * ECH, E - e0)
                    ps = mmpool.tile([128, 4 * ECH], f32, space="PSUM", tag="mm")
                    for s0 in range(0, bcnt, ECH):
                        cnt = min(ECH, bcnt - s0)
                        nc.tensor.matmul(
                            out=ps[:, s0:s0 + cnt],
                            lhsT=fw_sb[:, E + l * 128:E + (l + 1) * 128],
                            rhs=fw_sb[:, e0 + s0:e0 + s0 + cnt],
                            start=True,
                            stop=True,
                        )
                    nc.vector.tensor_copy(
                        out=tbl[:, 1 + e0:1 + e0 + bcnt],
                        in_=ps[:, :bcnt],
                    )
                for ch in range(NCHK):
                    gth = gpool.tile([128, PCH], f32, tag="gth")
                    c0 = IDXC[p] + ch * (PCH // 16)
                    nc.gpsimd.ap_gather(
                        out_ap=gth[:, :],
                        in_ap=tbl[:, :],
                        idxs_ap=idxw[:, c0:c0 + PCH // 16],
                        channels=128,
                        num_elems=BLK,
                        d=1,
                        num_idxs=PCH,
                    )
                    asl = acc[:, ch * PCH:(ch + 1) * PCH]
                    if p == 0:
                        nc.vector.tensor_copy(out=asl, in_=gth[:, :])
                    else:
                        nc.vector.tensor_add(out=asl, in0=asl, in1=gth[:, :])

            # 1/max(count,1) lives on channels c>=8; shift to value channels
            # tail, split by gather chunk so half overlaps the last gather
            rt = gpool.tile([128, PAIRS_G], f32, tag="gth")
            scratch = gpool.tile([128, PAIRS_G], f32, tag="gth")
            rt2 = apool.tile([128, PAIRS_G], f32)
            for ch in range(NCHK):
                sl = slice(ch * PCH, (ch + 1) * PCH)
                nc.vector.tensor_scalar_max(
                    out=rt[:, sl], in0=acc[:, sl], scalar1=1.0
                )
                # ~2 ULP, ~2.8x faster than InstReciprocal (counts in [1,5])
                nc.vector.reciprocal_approx_accurate(
                    out=rt2[:, sl], in_=rt[:, sl], scratch=scratch[:, sl]
                )
                nc.sync.dma_start(out=scratch[0:120, sl], in_=rt2[8:128, sl])
                nc.vector.tensor_tensor(
                    out=rt[0:120, sl], in0=acc[0:120, sl],
                    in1=scratch[0:120, sl], op=mybir.AluOpType.mult,
                )
                ni = PCH // N  # i-rows per chunk (4)
                for g in range(8):
                    i0 = g * IPG + ch * ni
                    dst = out[:, i0:i0 + ni, :]
                    nc.sync.dma_start(
                        out=dst.rearrange("h i j -> h i j"),
                        in_=rt[16 * g:16 * g + H, sl].rearrange(
                            "c (i j) -> c i j", j=N
                        ),
                    )
    nc.compile()
    return nc


def _host_prep(edge_features_s, edge_weights, shortest_path_edges):
    feats = np.asarray(edge_features_s, dtype=np.float32)
    ew = np.asarray(edge_weights, dtype=np.float32)
    spe = np.asarray(shortest_path_edges).astype(np.int64)

    # fw = [featsT17 | W_CH]:
    #   featsT17 [17, E]: feats^T with an appended ones row
    #   W_CH [17, 5*128]: col l*128+p: p%16<8 -> (W[l, p%16, :], 0); else (0.., 1)
    W = ew[1:L + 1].reshape(L, H, D)
    featsT17 = np.concatenate([feats.T, np.ones((1, E), np.float32)], axis=0)
    wch = np.zeros((D + 1, 5 * 128), np.float32)
    for l in range(L):
        for p in range(128):
            c = p % 16
            if c < H:
                wch[:D, l * 128 + p] = W[l, c]
            else:
                wch[D, l * 128 + p] = 1.0
    fw = np.ascontiguousarray(np.concatenate([featsT17, wch], axis=1))

    comb = (spe + 1).astype(np.int32)   # [N, N, L], 0 = padding
    # per-device wrapped index streams
    idxw_all = np.zeros((NCORES, 128, IDXW_COLS), np.int16)
    for cdev in range(NCORES):
        sub = comb[cdev * RPC:(cdev + 1) * RPC]  # [64, 512, 5]
        for p, ls in enumerate(PHASES):
            npos = PAIRS_G * len(ls)
            for g in range(8):
                st = sub[g * IPG:(g + 1) * IPG][:, :, ls].astype(np.int32)
                st = st + BLK * np.arange(len(ls), dtype=np.int32)
                flat = st.reshape(-1)
                wrapped = flat.reshape(npos // 16, 16).T
                idxw_all[cdev, 16 * g:16 * g + 16,
                         IDXC[p]:IDXC[p] + npos // 16] = wrapped
    return fw, idxw_all


def kernel(edge_features_s, edge_weights, shortest_path_edges):
    if "nc" not in _cached:
        _cached["nc"] = build_nc()
    nc = _cached["nc"]

    fw, idxw_all = _host_prep(edge_features_s, edge_weights, shortest_path_edges)
    in_maps = []
    for c in range(NCORES):
        in_maps.append({
            "idxw": np.ascontiguousarray(idxw_all[c]),
            "fw": fw,
        })
    res = run_bass_kernel_spmd(nc, in_maps, list(range(NCORES)))
    outs = [res.results[c]["out"].reshape(H, RPC, N) for c in range(NCORES)]
    return np.concatenate(outs, axis=1)

